# revision 18
# baseline (speedup 1.0000x reference)
"""FusedBitLinear Trainium2 kernel.

y = BitLinear(x, W, nw):
    rms   = sqrt(mean(x^2, -1) + 1e-6)
    x_n   = x / rms * nw
    alpha = max(mean(|W|), 1e-10)
    w_q   = clip(round(W / alpha), -1, 1)            (ternary)
    gamma = max(absmax(x_n, -1), 1e-10)
    x_q   = clip(round(x_n * 127 / gamma), -128, 127)
    y     = (x_q @ w_q.T) * (alpha * gamma / 127)

Key identities used on device:
    A[t]   = absmax(x[t,:] * nw)                     (per token)
    m[t]   = max(A[t], 1e-10 * rms[t])
    x_q    = round(x * nw * 127 / m[t])              (rms cancels; |..| <= 127 so
                                                      the clip never binds)
    y      = (x_q @ w_q.T) * alpha * m[t] / (127 * rms[t])
round() is the fp32 magic-add trick fused into an ACT fma (single rounding ->
exact round-to-nearest-even).  x_q in [-127,127] and ternary w_q are exact in
bf16, and 4096-long dot products of |v|<=127 integers fit fp32 PSUM exactly ->
the bf16 matmul is bit-exact.

Sharding (8 cores): 4 token-groups x 2 out-feature groups.  Each core gets
x rows [1024, 4096] and the k-major transpose of its W shard [4096, 2048].

Two launches: a tiny kernel reduces a disjoint 1/8 slice of |W| per core
(the only cross-core quantity), the host combines the 8 partials into
(1/alpha, alpha/127), and the main kernel takes those as a [1,2] input --
no collective on the main kernel's critical path.
"""

import numpy as np

import bass_rust as _bass_rust
import concourse.bass as bass
import concourse.mybir as mybir
import concourse.tile as tile
from concourse import bass_utils
from concourse.masks import make_identity
from concourse.vector_clock import ScopedClock, VectorClock

F32 = mybir.dt.float32
BF16 = mybir.dt.bfloat16
ALU = mybir.AluOpType
ACTF = mybir.ActivationFunctionType

N_CORES = 8
P = 128
K = 4096            # in_features
T_C = 1024          # tokens per core
O_C = 2048          # out features per core
N_T = T_C // P      # 8 token tiles
N_K = K // P        # 32 k tiles
OCW = 512           # out-feature chunk width (matmul moving free dim)
N_OC = O_C // OCW   # 4 chunks
TT_H = 4            # token tiles per half-group (psum banks per group)
MAGIC = 12582912.0  # 1.5 * 2**23 : fp32 round-to-nearest-even magic
NORM_EPS = 1e-6

_patched = False


def _patch_drain_and_barrier():
    """The walrus build in this env allows at most ~2 sync waits per
    instruction, but TileContext's exit drain piles one wait per logical
    processor onto a single Drain.  Split it: one drain per outstanding proc."""
    global _patched
    if _patched:
        return
    _patched = True

    def _drain_and_barrier(self, tick_clock, wait_clock):
        gvc = tick_clock.global_clock
        try:
            items = gvc.items()
        except AttributeError:
            items = [(None, gvc)]
        for scope, vc in items:
            for p in range(len(vc)):
                t = vc[p]
                if t <= 0:
                    continue
                part = VectorClock()
                part.require_at_least(p, t)
                d = self.nc.sync.drain()
                wait_clock.add_sem_waits(d.ins, ScopedClock({scope: part}))
        self.nc.all_engine_barrier()
        assert self.sems is not None
        popped = self.nc._tile_sem_poison_stack.pop()
        assert popped is self._sem_poison
        self.nc.clear_and_free_semaphores(list(self.sems.allocated().values()))
        self.nc.all_engine_barrier()

    tile.TileContext._drain_and_barrier = _drain_and_barrier


_MAX_WAITS = 1      # per-instruction wait slots walrus accepts (DMA: 1)
_EV_WAITS = 2       # EventSemaphore instructions can hold 2
_wsplit_n = [0]


def _split_excess_waits(nc: bass.Bass):
    """walrus rejects instructions with >1-2 sync waits.  Hoist the excess
    onto EventSemaphore instructions inserted immediately before, on the same
    engine (program order on that engine preserves the blocking semantics)."""
    for fn in nc.m.functions:
        for bb in fn.blocks:
            insts = bb.instructions
            out = []
            for ins in insts:
                si = ins.sync_info
                waits = list(si.on_wait) if si and si.on_wait else []
                if len(waits) > _MAX_WAITS:
                    keep = waits[-_MAX_WAITS:]
                    excess = waits[:-_MAX_WAITS]
                    for i in range(0, len(excess), _EV_WAITS):
                        ev = mybir.InstEventSemaphore(
                            name=f"wsplit-{_wsplit_n[0]}", ins=[], outs=[])
                        _wsplit_n[0] += 1
                        ev.engine = ins.engine
                        ev.sync_info = _bass_rust.SyncInfo(
                            on_wait=excess[i:i + _EV_WAITS], on_update=[])
                        out.append(ev)
                    ins.sync_info = _bass_rust.SyncInfo(
                        on_wait=keep,
                        on_update=list(si.on_update) if si.on_update else [])
                out.append(ins)
            insts[:] = out


def build_alpha_program() -> bass.Bass:
    """Per-core partial sum of |W| over a disjoint [1024, 2048] slice."""
    _patch_drain_and_barrier()
    nc = bass.Bass("TRN2", target_bir_lowering=False, debug=False,
                   enable_asserts=False, num_devices=N_CORES)
    wa = nc.dram_tensor("wa", [T_C, O_C], F32, kind="ExternalInput")
    ap_out = nc.dram_tensor("apart", [1, 1], F32, kind="ExternalOutput")
    wa_c = wa.ap().rearrange("(a b p) o -> a p b o", b=2, p=P)
    N_AC = 4
    with tile.TileContext(nc) as tc:
        with tc.tile_pool(name="sb", bufs=2) as sb, \
             tc.tile_pool(name="st", bufs=1) as st, \
             tc.tile_pool(name="ps", bufs=1, space="PSUM") as ps:
            ones_col = st.tile([P, 1], F32, name="ones_col")
            nc.gpsimd.memset(ones_col[:], 1.0)
            apart = st.tile([P, N_AC], F32, name="apart")
            for a in range(N_AC):
                at = sb.tile([P, 2, O_C], F32, name="aw")
                nc.sync.dma_start(at[:], wa_c[a])
                nc.vector.tensor_reduce(apart[:, a:a + 1], at[:],
                                        axis=mybir.AxisListType.XY, op=ALU.add,
                                        apply_absolute_value=True)
            asum = st.tile([P, 1], F32, name="asum")
            nc.vector.tensor_reduce(asum[:], apart[:],
                                    axis=mybir.AxisListType.X, op=ALU.add)
            psum_a = ps.tile([1, 1], F32, name="pss")
            nc.tensor.matmul(psum_a[:], lhsT=ones_col[:], rhs=asum[:],
                             start=True, stop=True)
            tsum = st.tile([1, 1], F32, name="tsum")
            nc.vector.tensor_copy(tsum[:], psum_a[:])
            nc.sync.dma_start(ap_out.ap(), tsum[:])
    _split_excess_waits(nc)
    return nc


def build_main_program(nw_ones: bool) -> bass.Bass:
    _patch_drain_and_barrier()
    nc = bass.Bass("TRN2", target_bir_lowering=False, debug=False,
                   enable_asserts=False, num_devices=N_CORES)
    xs = nc.dram_tensor("xs", [T_C, K], F32, kind="ExternalInput")
    wt = nc.dram_tensor("wt", [K, O_C], F32, kind="ExternalInput")
    abt = nc.dram_tensor("ab", [1, 2], F32, kind="ExternalInput")
    nwt = nc.dram_tensor("nw", [K], F32, kind="ExternalInput")
    ys = nc.dram_tensor("ys", [T_C, O_C], F32, kind="ExternalOutput")

    xs_r = xs.ap().rearrange("(a p) k -> a p k", p=P)
    wt_a = wt.ap()
    ys_a = ys.ap()

    with tile.TileContext(nc) as tc:
        with tc.tile_pool(name="const", bufs=1) as cst, \
             tc.tile_pool(name="stat", bufs=1) as st, \
             tc.tile_pool(name="xin", bufs=2) as xin_p, \
             tc.tile_pool(name="scr", bufs=1) as scr_p, \
             tc.tile_pool(name="xq", bufs=2) as xq_p, \
             tc.tile_pool(name="xqt", bufs=1) as xqt_p, \
             tc.tile_pool(name="wf", bufs=3) as wf_p, \
             tc.tile_pool(name="q1", bufs=3) as q1_p, \
             tc.tile_pool(name="wq", bufs=36) as wq_p, \
             tc.tile_pool(name="yo", bufs=3) as y_p, \
             tc.tile_pool(name="ptr", bufs=2, space="PSUM") as ptr_p, \
             tc.tile_pool(name="pacc", bufs=6, space="PSUM") as pacc_p:

            # ---------------- constants ----------------
            magic = cst.tile([P, 1], F32, name="magic")
            nc.gpsimd.memset(magic[:], MAGIC)
            epsc = cst.tile([P, 1], F32, name="epsc")
            nc.gpsimd.memset(epsc[:], NORM_EPS)
            ones_row = cst.tile([1, P], F32, name="ones_row")
            nc.gpsimd.memset(ones_row[:], 1.0)
            ident = cst.tile([P, P], BF16, name="ident")
            make_identity(nc, ident[:])

            # alpha scalars: ab = [1/alpha, alpha/127] -> broadcast to [128,2]
            ab_sb = cst.tile([1, 2], F32, name="ab_sb")
            nc.scalar.dma_start(ab_sb[:], abt.ap())
            psum_b = pacc_p.tile([P, OCW], F32, name="pacc")[:, 0:2]
            nc.tensor.matmul(psum_b[:], lhsT=ones_row[:], rhs=ab_sb[:],
                             start=True, stop=True)
            ab = st.tile([P, 2], F32, name="ab")
            nc.vector.tensor_copy(ab[:], psum_b[:])
            inv_a = ab[:, 0:1]   # [128,1] broadcast of 1/alpha
            al127 = ab[:, 1:2]   # [128,1] broadcast of alpha/127

            if not nw_ones:
                nw_b = cst.tile([P, K], F32, name="nw_b")
                nw_sb = cst.tile([1, K], F32, name="nw_sb")
                nc.scalar.dma_start(nw_sb[:], nwt.ap().rearrange("k -> 1 k"))
                for c in range(K // OCW):
                    pb = pacc_p.tile([P, OCW], F32, name="pacc")
                    nc.tensor.matmul(pb[:], lhsT=ones_row[:],
                                     rhs=nw_sb[:, c * OCW:(c + 1) * OCW],
                                     start=True, stop=True)
                    nc.vector.tensor_copy(nw_b[:, c * OCW:(c + 1) * OCW],
                                          pb[:])

            # ---------------- x pipeline ----------------
            xqt = xqt_p.tile([P, N_K, T_C], BF16, name="xqt")
            sy = [None] * N_T

            def x_phase(tt):
                xt = xin_p.tile([P, K], F32, name="xin")
                nc.sync.dma_start(xt[:], xs_r[tt])
                sq = scr_p.tile([P, K], BF16, name="scr")
                ssum = st.tile([P, 1], F32, name=f"ssum{tt}")
                nc.scalar.activation(sq[:], xt[:], ACTF.Square,
                                     accum_out=ssum[:])
                if not nw_ones:
                    nc.vector.tensor_tensor(xt[:], xt[:], nw_b[:], ALU.mult)
                amax = st.tile([P, 1], F32, name=f"amax{tt}")
                nc.vector.tensor_reduce(amax[:], xt[:],
                                        axis=mybir.AxisListType.X, op=ALU.max,
                                        apply_absolute_value=True)
                rms = st.tile([P, 1], F32, name=f"rms{tt}")
                nc.scalar.activation(rms[:], ssum[:], ACTF.Sqrt,
                                     scale=1.0 / K, bias=epsc[:])
                grd = st.tile([P, 1], F32, name=f"grd{tt}")
                nc.vector.tensor_scalar(grd[:], rms[:], 1e-10, None, ALU.mult)
                m = st.tile([P, 1], F32, name=f"m{tt}")
                nc.vector.tensor_tensor(m[:], amax[:], grd[:], ALU.max)
                m127 = st.tile([P, 1], F32, name=f"m127{tt}")
                nc.vector.tensor_scalar(m127[:], m[:], 1.0 / 127.0, None,
                                        ALU.mult)
                sA = st.tile([P, 1], F32, name=f"sA{tt}")
                nc.vector.reciprocal(sA[:], m127[:])
                # r = round(x * sA) + MAGIC   (in place over xt)
                nc.scalar.activation(xt[:], xt[:], ACTF.Identity,
                                     scale=sA[:], bias=magic[:])
                xq = xq_p.tile([P, K], BF16, name="xq")
                nc.gpsimd.tensor_scalar(xq[:], xt[:], MAGIC, None,
                                        ALU.subtract)
                for g in range(N_K // 4):
                    pst = ptr_p.tile([P, 4 * P], BF16, name="ptr")
                    for j in range(4):
                        kk = 4 * g + j
                        nc.tensor.transpose(pst[:, j * P:(j + 1) * P],
                                            xq[:, kk * P:(kk + 1) * P],
                                            ident[:])
                    nc.vector.tensor_copy(
                        xqt[:, 4 * g:4 * g + 4, tt * P:(tt + 1) * P],
                        pst[:].rearrange("p (j c) -> p j c", j=4))
                # S_y = alpha * m / (127 * rms)
                rinv = st.tile([P, 1], F32, name=f"rinv{tt}")
                nc.vector.reciprocal(rinv[:], rms[:])
                t1 = st.tile([P, 1], F32, name=f"t1{tt}")
                nc.vector.tensor_scalar(t1[:], m[:], al127, None, ALU.mult)
                syt = st.tile([P, 1], F32, name=f"sy{tt}")
                nc.vector.tensor_tensor(syt[:], t1[:], rinv[:], ALU.mult)
                sy[tt] = syt

            def w_quant(oc):
                tiles = []
                for kk in range(N_K):
                    wf = wf_p.tile([P, OCW], F32, name="wf")
                    nc.sync.dma_start(
                        wf[:], wt_a[kk * P:(kk + 1) * P,
                                    oc * OCW:(oc + 1) * OCW])
                    # r = round(w / alpha) + MAGIC (in place).  Alternate the
                    # round between ACT and DVE so wq production outpaces the
                    # PE's consumption (ACT alone is the choke point).
                    if kk % 2 == 0:
                        nc.scalar.activation(wf[:], wf[:], ACTF.Identity,
                                             scale=inv_a, bias=magic[:])
                    else:
                        nc.vector.tensor_scalar(wf[:], wf[:], inv_a, MAGIC,
                                                ALU.mult, ALU.add)
                    q1 = q1_p.tile([P, OCW], BF16, name="q1")
                    nc.vector.tensor_scalar(q1[:], wf[:], MAGIC, 1.0,
                                            ALU.subtract, ALU.min)
                    wq = wq_p.tile([P, OCW], BF16, name="wq")
                    nc.vector.tensor_scalar(wq[:], q1[:], -1.0, None, ALU.max)
                    tiles.append(wq)
                return tiles

            def mm_phase(oc, wq_tiles):
                # kk-inner over tt half-groups: each wq tile's last reader is
                # early in the chunk, so the next chunk's quant runs ahead.
                for h in range(N_T // TT_H):
                    tts = list(range(h * TT_H, (h + 1) * TT_H))
                    pas = {tt: pacc_p.tile([P, OCW], F32, name="pacc")
                           for tt in tts}
                    for kk in range(N_K):
                        for tt in tts:
                            nc.tensor.matmul(
                                pas[tt][:],
                                lhsT=xqt[:, kk, tt * P:(tt + 1) * P],
                                rhs=wq_tiles[kk][:],
                                start=(kk == 0), stop=(kk == N_K - 1))
                    for tt in tts:
                        yt = y_p.tile([P, OCW], F32, name="yo")
                        nc.scalar.activation(yt[:], pas[tt][:], ACTF.Identity,
                                             scale=sy[tt][:])
                        nc.sync.dma_start(
                            ys_a[tt * P:(tt + 1) * P,
                                 oc * OCW:(oc + 1) * OCW],
                            yt[:])

            # Emission order drives scheduling priority + DMA queue order.
            x_phase(0)
            wq_cur = w_quant(0)
            for tt in range(1, N_T):
                x_phase(tt)
            for oc in range(N_OC):
                mm_phase(oc, wq_cur)
                if oc + 1 < N_OC:
                    wq_cur = w_quant(oc + 1)
    _split_excess_waits(nc)
    return nc


_PROGRAMS: dict = {}


def _get_program(key):
    if key not in _PROGRAMS:
        if key == "alpha":
            _PROGRAMS[key] = build_alpha_program()
        else:
            _PROGRAMS[key] = build_main_program(key == "main_ones")
    return _PROGRAMS[key]


def kernel(x, weight, norm_weight, _trace=False, _trace_kwargs=None):
    x = np.ascontiguousarray(np.asarray(x, dtype=np.float32))
    W = np.asarray(weight, dtype=np.float32)
    nw = np.ascontiguousarray(np.asarray(norm_weight, dtype=np.float32))
    b, s, k = x.shape
    assert (b * s, k) == (4096, K) and W.shape == (4096, K)
    x2 = x.reshape(b * s, k)
    nw_ones = bool(np.all(nw == 1.0))
    # k-major shards of W (layout prep only -- no arithmetic)
    wts = [np.ascontiguousarray(W[O_C * j:O_C * (j + 1), :].T)
           for j in range(2)]

    kwargs = dict(trace=True, **(_trace_kwargs or {})) if _trace else {}

    # ---- launch 1: alpha partials over disjoint 1/8 slices of W ----
    nc_a = _get_program("alpha")
    in_a = []
    for c in range(N_CORES):
        i, j = c % 4, c // 4
        in_a.append({"wa": wts[j][T_C * i:T_C * (i + 1)]})
    res_a = bass_utils.run_bass_kernel_spmd(
        nc_a, in_a, core_ids=list(range(N_CORES)), **kwargs)
    total = np.float64(0.0)
    for c in range(N_CORES):
        total += np.float64(res_a.results[c]["apart"][0, 0])
    alpha = np.maximum(np.float32(np.float32(total) / np.float32(K * 4096)),
                       np.float32(1e-10))
    ab = np.array([[np.float32(1.0) / alpha, alpha / np.float32(127.0)]],
                  dtype=np.float32)

    # ---- launch 2: main kernel ----
    nc_m = _get_program("main_ones" if nw_ones else "main_gen")
    in_m = []
    for c in range(N_CORES):
        i, j = c % 4, c // 4
        in_m.append({"xs": x2[T_C * i:T_C * (i + 1)], "wt": wts[j],
                     "ab": ab, "nw": nw})
    res_m = bass_utils.run_bass_kernel_spmd(
        nc_m, in_m, core_ids=list(range(N_CORES)), **kwargs)

    y = np.empty((4096, 4096), dtype=np.float32)
    for c in range(N_CORES):
        i, j = c % 4, c // 4
        y[T_C * i:T_C * (i + 1), O_C * j:O_C * (j + 1)] = \
            res_m.results[c]["ys"]
    out = y.reshape(b, s, 4096)
    if _trace:
        return out, (res_a, res_m)
    return out


# revision 19
# speedup vs baseline: 2.0244x; 2.0244x over previous
"""FusedBitLinear Trainium2 kernel.

y = BitLinear(x, W, nw):
    rms   = sqrt(mean(x^2, -1) + 1e-6)
    x_n   = x / rms * nw
    alpha = max(mean(|W|), 1e-10)
    w_q   = clip(round(W / alpha), -1, 1)            (ternary)
    gamma = max(absmax(x_n, -1), 1e-10)
    x_q   = clip(round(x_n * 127 / gamma), -128, 127)
    y     = (x_q @ w_q.T) * (alpha * gamma / 127)

Key identities used on device:
    A[t]   = absmax(x[t,:] * nw)                     (per token)
    m[t]   = max(A[t], 1e-10 * rms[t])
    x_q    = round(x * nw * 127 / m[t])              (rms cancels; |..| <= 127 so
                                                      the clip never binds)
    y      = (x_q @ w_q.T) * alpha * m[t] / (127 * rms[t])
round() is the fp32 magic-add trick fused into an ACT fma (single rounding ->
exact round-to-nearest-even).  x_q in [-127,127] and ternary w_q are exact in
bf16, and 4096-long dot products of |v|<=127 integers fit fp32 PSUM exactly ->
the bf16 matmul is bit-exact.

Sharding (8 cores): 4 token-groups x 2 out-feature groups.  Each core gets
x rows [1024, 4096] and the k-major transpose of its W shard [4096, 2048].

Two launches: a tiny kernel reduces a disjoint 1/8 slice of |W| per core
(the only cross-core quantity), the host combines the 8 partials into
(1/alpha, alpha/127), and the main kernel takes those as a [1,2] input --
no collective on the main kernel's critical path.
"""

import numpy as np

import bass_rust as _bass_rust
import concourse.bass as bass
import concourse.mybir as mybir
import concourse.tile as tile
from concourse import bass_utils
from concourse.masks import make_identity
from concourse.vector_clock import ScopedClock, VectorClock

F32 = mybir.dt.float32
BF16 = mybir.dt.bfloat16
ALU = mybir.AluOpType
ACTF = mybir.ActivationFunctionType

N_CORES = 8
P = 128
K = 4096            # in_features
T_C = 1024          # tokens per core
O_C = 2048          # out features per core
N_T = T_C // P      # 8 token tiles
N_K = K // P        # 32 k tiles
OCW = 512           # out-feature chunk width (matmul moving free dim)
N_OC = O_C // OCW   # 4 chunks
TT_H = 4            # token tiles per half-group (psum banks per group)
MAGIC = 12582912.0  # 1.5 * 2**23 : fp32 round-to-nearest-even magic
NORM_EPS = 1e-6

_patched = False


def _patch_drain_and_barrier():
    """The walrus build in this env allows at most ~2 sync waits per
    instruction, but TileContext's exit drain piles one wait per logical
    processor onto a single Drain.  Split it: one drain per outstanding proc."""
    global _patched
    if _patched:
        return
    _patched = True

    def _drain_and_barrier(self, tick_clock, wait_clock):
        gvc = tick_clock.global_clock
        try:
            items = gvc.items()
        except AttributeError:
            items = [(None, gvc)]
        for scope, vc in items:
            for p in range(len(vc)):
                t = vc[p]
                if t <= 0:
                    continue
                part = VectorClock()
                part.require_at_least(p, t)
                d = self.nc.sync.drain()
                wait_clock.add_sem_waits(d.ins, ScopedClock({scope: part}))
        self.nc.all_engine_barrier()
        assert self.sems is not None
        popped = self.nc._tile_sem_poison_stack.pop()
        assert popped is self._sem_poison
        self.nc.clear_and_free_semaphores(list(self.sems.allocated().values()))
        self.nc.all_engine_barrier()

    tile.TileContext._drain_and_barrier = _drain_and_barrier


_MAX_WAITS = 1      # per-instruction wait slots walrus accepts (DMA: 1)
_EV_WAITS = 2       # EventSemaphore instructions can hold 2
_wsplit_n = [0]


def _split_excess_waits(nc: bass.Bass):
    """walrus rejects instructions with >1-2 sync waits.  Hoist the excess
    onto EventSemaphore instructions inserted immediately before, on the same
    engine (program order on that engine preserves the blocking semantics)."""
    for fn in nc.m.functions:
        for bb in fn.blocks:
            insts = bb.instructions
            out = []
            for ins in insts:
                si = ins.sync_info
                waits = list(si.on_wait) if si and si.on_wait else []
                if len(waits) > _MAX_WAITS:
                    keep = waits[-_MAX_WAITS:]
                    excess = waits[:-_MAX_WAITS]
                    for i in range(0, len(excess), _EV_WAITS):
                        ev = mybir.InstEventSemaphore(
                            name=f"wsplit-{_wsplit_n[0]}", ins=[], outs=[])
                        _wsplit_n[0] += 1
                        ev.engine = ins.engine
                        ev.sync_info = _bass_rust.SyncInfo(
                            on_wait=excess[i:i + _EV_WAITS], on_update=[])
                        out.append(ev)
                    ins.sync_info = _bass_rust.SyncInfo(
                        on_wait=keep,
                        on_update=list(si.on_update) if si.on_update else [])
                out.append(ins)
            insts[:] = out


def build_alpha_program() -> bass.Bass:
    """Per-core partial sum of |W| over a disjoint [1024, 2048] slice."""
    _patch_drain_and_barrier()
    nc = bass.Bass("TRN2", target_bir_lowering=False, debug=False,
                   enable_asserts=False, num_devices=N_CORES)
    wa = nc.dram_tensor("wa", [T_C, O_C], F32, kind="ExternalInput")
    ap_out = nc.dram_tensor("apart", [1, 1], F32, kind="ExternalOutput")
    wa_c = wa.ap().rearrange("(a b p) o -> a p b o", b=2, p=P)
    N_AC = 4
    with tile.TileContext(nc) as tc:
        with tc.tile_pool(name="sb", bufs=2) as sb, \
             tc.tile_pool(name="st", bufs=1) as st, \
             tc.tile_pool(name="ps", bufs=1, space="PSUM") as ps:
            ones_col = st.tile([P, 1], F32, name="ones_col")
            nc.gpsimd.memset(ones_col[:], 1.0)
            apart = st.tile([P, N_AC], F32, name="apart")
            for a in range(N_AC):
                at = sb.tile([P, 2, O_C], F32, name="aw")
                nc.sync.dma_start(at[:], wa_c[a])
                nc.vector.tensor_reduce(apart[:, a:a + 1], at[:],
                                        axis=mybir.AxisListType.XY, op=ALU.add,
                                        apply_absolute_value=True)
            asum = st.tile([P, 1], F32, name="asum")
            nc.vector.tensor_reduce(asum[:], apart[:],
                                    axis=mybir.AxisListType.X, op=ALU.add)
            psum_a = ps.tile([1, 1], F32, name="pss")
            nc.tensor.matmul(psum_a[:], lhsT=ones_col[:], rhs=asum[:],
                             start=True, stop=True)
            tsum = st.tile([1, 1], F32, name="tsum")
            nc.vector.tensor_copy(tsum[:], psum_a[:])
            nc.sync.dma_start(ap_out.ap(), tsum[:])
    _split_excess_waits(nc)
    return nc


def build_main_program(nw_ones: bool) -> bass.Bass:
    _patch_drain_and_barrier()
    nc = bass.Bass("TRN2", target_bir_lowering=False, debug=False,
                   enable_asserts=False, num_devices=N_CORES)
    xs = nc.dram_tensor("xs", [T_C, K], F32, kind="ExternalInput")
    wt = nc.dram_tensor("wt", [K, O_C], F32, kind="ExternalInput")
    abt = nc.dram_tensor("ab", [1, 2], F32, kind="ExternalInput")
    nwt = nc.dram_tensor("nw", [K], F32, kind="ExternalInput")
    ys = nc.dram_tensor("ys", [T_C, O_C], F32, kind="ExternalOutput")

    xs_r = xs.ap().rearrange("(a p) k -> a p k", p=P)
    wt_a = wt.ap()
    ys_a = ys.ap()

    with tile.TileContext(nc) as tc:
        with tc.tile_pool(name="const", bufs=1) as cst, \
             tc.tile_pool(name="stat", bufs=1) as st, \
             tc.tile_pool(name="xin", bufs=2) as xin_p, \
             tc.tile_pool(name="scr", bufs=1) as scr_p, \
             tc.tile_pool(name="xq", bufs=2) as xq_p, \
             tc.tile_pool(name="xqt", bufs=1) as xqt_p, \
             tc.tile_pool(name="wf", bufs=3) as wf_p, \
             tc.tile_pool(name="q1", bufs=3) as q1_p, \
             tc.tile_pool(name="wq", bufs=36) as wq_p, \
             tc.tile_pool(name="yo", bufs=3) as y_p, \
             tc.tile_pool(name="ptr", bufs=2, space="PSUM") as ptr_p, \
             tc.tile_pool(name="pacc", bufs=6, space="PSUM") as pacc_p:

            # ---------------- constants ----------------
            magic = cst.tile([P, 1], F32, name="magic")
            nc.gpsimd.memset(magic[:], MAGIC)
            epsc = cst.tile([P, 1], F32, name="epsc")
            nc.gpsimd.memset(epsc[:], NORM_EPS)
            ones_row = cst.tile([1, P], F32, name="ones_row")
            nc.gpsimd.memset(ones_row[:], 1.0)
            ident = cst.tile([P, P], BF16, name="ident")
            make_identity(nc, ident[:])

            # alpha scalars: ab = [1/alpha, alpha/127] -> broadcast to [128,2]
            ab_sb = cst.tile([1, 2], F32, name="ab_sb")
            nc.scalar.dma_start(ab_sb[:], abt.ap())
            psum_b = pacc_p.tile([P, OCW], F32, name="pacc")[:, 0:2]
            nc.tensor.matmul(psum_b[:], lhsT=ones_row[:], rhs=ab_sb[:],
                             start=True, stop=True)
            ab = st.tile([P, 2], F32, name="ab")
            nc.vector.tensor_copy(ab[:], psum_b[:])
            inv_a = ab[:, 0:1]   # [128,1] broadcast of 1/alpha
            al127 = ab[:, 1:2]   # [128,1] broadcast of alpha/127

            if not nw_ones:
                nw_b = cst.tile([P, K], F32, name="nw_b")
                nw_sb = cst.tile([1, K], F32, name="nw_sb")
                nc.scalar.dma_start(nw_sb[:], nwt.ap().rearrange("k -> 1 k"))
                for c in range(K // OCW):
                    pb = pacc_p.tile([P, OCW], F32, name="pacc")
                    nc.tensor.matmul(pb[:], lhsT=ones_row[:],
                                     rhs=nw_sb[:, c * OCW:(c + 1) * OCW],
                                     start=True, stop=True)
                    nc.vector.tensor_copy(nw_b[:, c * OCW:(c + 1) * OCW],
                                          pb[:])

            # ---------------- x pipeline ----------------
            xqt = xqt_p.tile([P, N_K, T_C], BF16, name="xqt")
            sy = [None] * N_T

            def x_phase(tt):
                xt = xin_p.tile([P, K], F32, name="xin")
                nc.sync.dma_start(xt[:], xs_r[tt])
                sq = scr_p.tile([P, K], BF16, name="scr")
                ssum = st.tile([P, 1], F32, name=f"ssum{tt}")
                nc.scalar.activation(sq[:], xt[:], ACTF.Square,
                                     accum_out=ssum[:])
                if not nw_ones:
                    nc.vector.tensor_tensor(xt[:], xt[:], nw_b[:], ALU.mult)
                amax = st.tile([P, 1], F32, name=f"amax{tt}")
                nc.vector.tensor_reduce(amax[:], xt[:],
                                        axis=mybir.AxisListType.X, op=ALU.max,
                                        apply_absolute_value=True)
                rms = st.tile([P, 1], F32, name=f"rms{tt}")
                nc.scalar.activation(rms[:], ssum[:], ACTF.Sqrt,
                                     scale=1.0 / K, bias=epsc[:])
                grd = st.tile([P, 1], F32, name=f"grd{tt}")
                nc.vector.tensor_scalar(grd[:], rms[:], 1e-10, None, ALU.mult)
                m = st.tile([P, 1], F32, name=f"m{tt}")
                nc.vector.tensor_tensor(m[:], amax[:], grd[:], ALU.max)
                m127 = st.tile([P, 1], F32, name=f"m127{tt}")
                nc.vector.tensor_scalar(m127[:], m[:], 1.0 / 127.0, None,
                                        ALU.mult)
                sA = st.tile([P, 1], F32, name=f"sA{tt}")
                nc.vector.reciprocal(sA[:], m127[:])
                # r = round(x * sA) + MAGIC   (in place over xt)
                nc.scalar.activation(xt[:], xt[:], ACTF.Identity,
                                     scale=sA[:], bias=magic[:])
                xq = xq_p.tile([P, K], BF16, name="xq")
                nc.vector.tensor_scalar(xq[:], xt[:], MAGIC, None,
                                        ALU.subtract)
                for g in range(N_K // 4):
                    pst = ptr_p.tile([P, 4 * P], BF16, name="ptr")
                    for j in range(4):
                        kk = 4 * g + j
                        nc.tensor.transpose(pst[:, j * P:(j + 1) * P],
                                            xq[:, kk * P:(kk + 1) * P],
                                            ident[:])
                    nc.vector.tensor_copy(
                        xqt[:, 4 * g:4 * g + 4, tt * P:(tt + 1) * P],
                        pst[:].rearrange("p (j c) -> p j c", j=4))
                # S_y = alpha * m / (127 * rms)
                rinv = st.tile([P, 1], F32, name=f"rinv{tt}")
                nc.vector.reciprocal(rinv[:], rms[:])
                t1 = st.tile([P, 1], F32, name=f"t1{tt}")
                nc.vector.tensor_scalar(t1[:], m[:], al127, None, ALU.mult)
                syt = st.tile([P, 1], F32, name=f"sy{tt}")
                nc.vector.tensor_tensor(syt[:], t1[:], rinv[:], ALU.mult)
                sy[tt] = syt

            def w_quant_one(oc, kk):
                wf = wf_p.tile([P, OCW], F32, name="wf")
                nc.sync.dma_start(
                    wf[:], wt_a[kk * P:(kk + 1) * P,
                                oc * OCW:(oc + 1) * OCW])
                # r = round(w / alpha) + MAGIC   (in place)
                nc.scalar.activation(wf[:], wf[:], ACTF.Identity,
                                     scale=inv_a, bias=magic[:])
                q1 = q1_p.tile([P, OCW], BF16, name="q1")
                nc.vector.tensor_scalar(q1[:], wf[:], MAGIC, 1.0,
                                        ALU.subtract, ALU.min)
                wq = wq_p.tile([P, OCW], BF16, name="wq")
                nc.vector.tensor_scalar(wq[:], q1[:], -1.0, None, ALU.max)
                return wq

            def mm_phase(oc, wq_tiles):
                # kk-inner over tt half-groups: each wq tile's last reader is
                # early in the chunk.  During the second half-group, interleave
                # the NEXT chunk's quant chain right behind each freed slot so
                # wq production stays phase-aligned with consumption.
                nxt = []
                for h in range(N_T // TT_H):
                    tts = list(range(h * TT_H, (h + 1) * TT_H))
                    pas = {tt: pacc_p.tile([P, OCW], F32, name="pacc")
                           for tt in tts}
                    for kk in range(N_K):
                        for tt in tts:
                            nc.tensor.matmul(
                                pas[tt][:],
                                lhsT=xqt[:, kk, tt * P:(tt + 1) * P],
                                rhs=wq_tiles[kk][:],
                                start=(kk == 0), stop=(kk == N_K - 1))
                        if h == 1 and oc + 1 < N_OC:
                            nxt.append(w_quant_one(oc + 1, kk))
                    for tt in tts:
                        yt = y_p.tile([P, OCW], F32, name="yo")
                        nc.scalar.activation(yt[:], pas[tt][:], ACTF.Identity,
                                             scale=sy[tt][:])
                        nc.sync.dma_start(
                            ys_a[tt * P:(tt + 1) * P,
                                 oc * OCW:(oc + 1) * OCW],
                            yt[:])
                return nxt

            # Emission order drives scheduling priority + DMA queue order.
            x_phase(0)
            wq_cur = [w_quant_one(0, kk) for kk in range(N_K)]
            for tt in range(1, N_T):
                x_phase(tt)
            for oc in range(N_OC):
                wq_cur = mm_phase(oc, wq_cur)
    _split_excess_waits(nc)
    return nc


_PROGRAMS: dict = {}


def _get_program(key):
    if key not in _PROGRAMS:
        if key == "alpha":
            _PROGRAMS[key] = build_alpha_program()
        else:
            _PROGRAMS[key] = build_main_program(key == "main_ones")
    return _PROGRAMS[key]


def kernel(x, weight, norm_weight, _trace=False, _trace_kwargs=None):
    x = np.ascontiguousarray(np.asarray(x, dtype=np.float32))
    W = np.asarray(weight, dtype=np.float32)
    nw = np.ascontiguousarray(np.asarray(norm_weight, dtype=np.float32))
    b, s, k = x.shape
    assert (b * s, k) == (4096, K) and W.shape == (4096, K)
    x2 = x.reshape(b * s, k)
    nw_ones = bool(np.all(nw == 1.0))
    # k-major shards of W (layout prep only -- no arithmetic)
    wts = [np.ascontiguousarray(W[O_C * j:O_C * (j + 1), :].T)
           for j in range(2)]

    kwargs = dict(trace=True, **(_trace_kwargs or {})) if _trace else {}

    # ---- launch 1: alpha partials over disjoint 1/8 slices of W ----
    nc_a = _get_program("alpha")
    in_a = []
    for c in range(N_CORES):
        i, j = c % 4, c // 4
        in_a.append({"wa": wts[j][T_C * i:T_C * (i + 1)]})
    res_a = bass_utils.run_bass_kernel_spmd(
        nc_a, in_a, core_ids=list(range(N_CORES)), **kwargs)
    total = np.float64(0.0)
    for c in range(N_CORES):
        total += np.float64(res_a.results[c]["apart"][0, 0])
    alpha = np.maximum(np.float32(np.float32(total) / np.float32(K * 4096)),
                       np.float32(1e-10))
    ab = np.array([[np.float32(1.0) / alpha, alpha / np.float32(127.0)]],
                  dtype=np.float32)

    # ---- launch 2: main kernel ----
    nc_m = _get_program("main_ones" if nw_ones else "main_gen")
    in_m = []
    for c in range(N_CORES):
        i, j = c % 4, c // 4
        in_m.append({"xs": x2[T_C * i:T_C * (i + 1)], "wt": wts[j],
                     "ab": ab, "nw": nw})
    res_m = bass_utils.run_bass_kernel_spmd(
        nc_m, in_m, core_ids=list(range(N_CORES)), **kwargs)

    y = np.empty((4096, 4096), dtype=np.float32)
    for c in range(N_CORES):
        i, j = c % 4, c // 4
        y[T_C * i:T_C * (i + 1), O_C * j:O_C * (j + 1)] = \
            res_m.results[c]["ys"]
    out = y.reshape(b, s, 4096)
    if _trace:
        return out, (res_a, res_m)
    return out


# revision 20
# speedup vs baseline: 2.1562x; 1.0651x over previous
"""FusedBitLinear Trainium2 kernel.

y = BitLinear(x, W, nw):
    rms   = sqrt(mean(x^2, -1) + 1e-6)
    x_n   = x / rms * nw
    alpha = max(mean(|W|), 1e-10)
    w_q   = clip(round(W / alpha), -1, 1)            (ternary)
    gamma = max(absmax(x_n, -1), 1e-10)
    x_q   = clip(round(x_n * 127 / gamma), -128, 127)
    y     = (x_q @ w_q.T) * (alpha * gamma / 127)

Key identities used on device:
    A[t]   = absmax(x[t,:] * nw)                     (per token)
    m[t]   = max(A[t], 1e-10 * rms[t])
    x_q    = round(x * nw * 127 / m[t])              (rms cancels; |..| <= 127 so
                                                      the clip never binds)
    y      = (x_q @ w_q.T) * alpha * m[t] / (127 * rms[t])
round() is the fp32 magic-add trick fused into an ACT fma (single rounding ->
exact round-to-nearest-even).  x_q in [-127,127] and ternary w_q are exact in
bf16, and 4096-long dot products of |v|<=127 integers fit fp32 PSUM exactly ->
the bf16 matmul is bit-exact.

Sharding (8 cores): 4 token-groups x 2 out-feature groups.  Each core gets
x rows [1024, 4096] and the k-major transpose of its W shard [4096, 2048].

Two launches: a tiny kernel reduces a disjoint 1/8 slice of |W| per core
(the only cross-core quantity), the host combines the 8 partials into
(1/alpha, alpha/127), and the main kernel takes those as a [1,2] input --
no collective on the main kernel's critical path.
"""

import numpy as np

import bass_rust as _bass_rust
import concourse.bass as bass
import concourse.mybir as mybir
import concourse.tile as tile
from concourse import bass_utils
from concourse.masks import make_identity
from concourse.vector_clock import ScopedClock, VectorClock

F32 = mybir.dt.float32
BF16 = mybir.dt.bfloat16
ALU = mybir.AluOpType
ACTF = mybir.ActivationFunctionType

N_CORES = 8
P = 128
K = 4096            # in_features
T_C = 1024          # tokens per core
O_C = 2048          # out features per core
N_T = T_C // P      # 8 token tiles
N_K = K // P        # 32 k tiles
OCW = 512           # out-feature chunk width (matmul moving free dim)
N_OC = O_C // OCW   # 4 chunks
TT_H = 4            # token tiles per half-group (psum banks per group)
MAGIC = 12582912.0  # 1.5 * 2**23 : fp32 round-to-nearest-even magic
NORM_EPS = 1e-6

_patched = False


def _patch_drain_and_barrier():
    """The walrus build in this env allows at most ~2 sync waits per
    instruction, but TileContext's exit drain piles one wait per logical
    processor onto a single Drain.  Split it: one drain per outstanding proc."""
    global _patched
    if _patched:
        return
    _patched = True

    def _drain_and_barrier(self, tick_clock, wait_clock):
        gvc = tick_clock.global_clock
        try:
            items = gvc.items()
        except AttributeError:
            items = [(None, gvc)]
        for scope, vc in items:
            for p in range(len(vc)):
                t = vc[p]
                if t <= 0:
                    continue
                part = VectorClock()
                part.require_at_least(p, t)
                d = self.nc.sync.drain()
                wait_clock.add_sem_waits(d.ins, ScopedClock({scope: part}))
        self.nc.all_engine_barrier()
        assert self.sems is not None
        popped = self.nc._tile_sem_poison_stack.pop()
        assert popped is self._sem_poison
        self.nc.clear_and_free_semaphores(list(self.sems.allocated().values()))
        self.nc.all_engine_barrier()

    tile.TileContext._drain_and_barrier = _drain_and_barrier


_MAX_WAITS = 1      # per-instruction wait slots walrus accepts (DMA: 1)
_EV_WAITS = 2       # EventSemaphore instructions can hold 2
_wsplit_n = [0]


def _split_excess_waits(nc: bass.Bass):
    """walrus rejects instructions with >1-2 sync waits.  Hoist the excess
    onto EventSemaphore instructions inserted immediately before, on the same
    engine (program order on that engine preserves the blocking semantics)."""
    for fn in nc.m.functions:
        for bb in fn.blocks:
            insts = bb.instructions
            out = []
            for ins in insts:
                si = ins.sync_info
                waits = list(si.on_wait) if si and si.on_wait else []
                if len(waits) > _MAX_WAITS:
                    keep = waits[-_MAX_WAITS:]
                    excess = waits[:-_MAX_WAITS]
                    for i in range(0, len(excess), _EV_WAITS):
                        ev = mybir.InstEventSemaphore(
                            name=f"wsplit-{_wsplit_n[0]}", ins=[], outs=[])
                        _wsplit_n[0] += 1
                        ev.engine = ins.engine
                        ev.sync_info = _bass_rust.SyncInfo(
                            on_wait=excess[i:i + _EV_WAITS], on_update=[])
                        out.append(ev)
                    ins.sync_info = _bass_rust.SyncInfo(
                        on_wait=keep,
                        on_update=list(si.on_update) if si.on_update else [])
                out.append(ins)
            insts[:] = out


def build_alpha_program() -> bass.Bass:
    """Per-core partial sum of |W| over a disjoint [1024, 2048] slice."""
    _patch_drain_and_barrier()
    nc = bass.Bass("TRN2", target_bir_lowering=False, debug=False,
                   enable_asserts=False, num_devices=N_CORES)
    wa = nc.dram_tensor("wa", [T_C, O_C], F32, kind="ExternalInput")
    ap_out = nc.dram_tensor("apart", [1, 1], F32, kind="ExternalOutput")
    wa_c = wa.ap().rearrange("(a b p) o -> a p b o", b=2, p=P)
    N_AC = 4
    with tile.TileContext(nc) as tc:
        with tc.tile_pool(name="sb", bufs=2) as sb, \
             tc.tile_pool(name="st", bufs=1) as st, \
             tc.tile_pool(name="ps", bufs=1, space="PSUM") as ps:
            ones_col = st.tile([P, 1], F32, name="ones_col")
            nc.gpsimd.memset(ones_col[:], 1.0)
            apart = st.tile([P, N_AC], F32, name="apart")
            for a in range(N_AC):
                at = sb.tile([P, 2, O_C], F32, name="aw")
                nc.sync.dma_start(at[:], wa_c[a])
                nc.vector.tensor_reduce(apart[:, a:a + 1], at[:],
                                        axis=mybir.AxisListType.XY, op=ALU.add,
                                        apply_absolute_value=True)
            asum = st.tile([P, 1], F32, name="asum")
            nc.vector.tensor_reduce(asum[:], apart[:],
                                    axis=mybir.AxisListType.X, op=ALU.add)
            psum_a = ps.tile([1, 1], F32, name="pss")
            nc.tensor.matmul(psum_a[:], lhsT=ones_col[:], rhs=asum[:],
                             start=True, stop=True)
            tsum = st.tile([1, 1], F32, name="tsum")
            nc.vector.tensor_copy(tsum[:], psum_a[:])
            nc.sync.dma_start(ap_out.ap(), tsum[:])
    _split_excess_waits(nc)
    return nc


def build_main_program(nw_ones: bool) -> bass.Bass:
    _patch_drain_and_barrier()
    nc = bass.Bass("TRN2", target_bir_lowering=False, debug=False,
                   enable_asserts=False, num_devices=N_CORES)
    xs = nc.dram_tensor("xs", [T_C, K], F32, kind="ExternalInput")
    wt = nc.dram_tensor("wt", [K, O_C], F32, kind="ExternalInput")
    abt = nc.dram_tensor("ab", [1, 2], F32, kind="ExternalInput")
    nwt = nc.dram_tensor("nw", [K], F32, kind="ExternalInput")
    ys = nc.dram_tensor("ys", [T_C, O_C], F32, kind="ExternalOutput")

    xs_r = xs.ap().rearrange("(a p) k -> a p k", p=P)
    wt_a = wt.ap()
    ys_a = ys.ap()

    with tile.TileContext(nc) as tc:
        with tc.tile_pool(name="const", bufs=1) as cst, \
             tc.tile_pool(name="stat", bufs=1) as st, \
             tc.tile_pool(name="xin", bufs=2) as xin_p, \
             tc.tile_pool(name="scr", bufs=1) as scr_p, \
             tc.tile_pool(name="xq", bufs=2) as xq_p, \
             tc.tile_pool(name="xqt", bufs=1) as xqt_p, \
             tc.tile_pool(name="wf", bufs=3) as wf_p, \
             tc.tile_pool(name="q1", bufs=3) as q1_p, \
             tc.tile_pool(name="wq", bufs=18) as wq_p, \
             tc.tile_pool(name="yo", bufs=3) as y_p, \
             tc.tile_pool(name="ptr", bufs=2, space="PSUM") as ptr_p, \
             tc.tile_pool(name="pacc", bufs=6, space="PSUM") as pacc_p:

            # ---------------- constants ----------------
            magic = cst.tile([P, 1], F32, name="magic")
            nc.gpsimd.memset(magic[:], MAGIC)
            epsc = cst.tile([P, 1], F32, name="epsc")
            nc.gpsimd.memset(epsc[:], NORM_EPS)
            ones_row = cst.tile([1, P], F32, name="ones_row")
            nc.gpsimd.memset(ones_row[:], 1.0)
            ident = cst.tile([P, P], BF16, name="ident")
            make_identity(nc, ident[:])

            # alpha scalars: ab = [1/alpha, alpha/127] -> broadcast to [128,2]
            ab_sb = cst.tile([1, 2], F32, name="ab_sb")
            nc.scalar.dma_start(ab_sb[:], abt.ap())
            psum_b = pacc_p.tile([P, OCW], F32, name="pacc")[:, 0:2]
            nc.tensor.matmul(psum_b[:], lhsT=ones_row[:], rhs=ab_sb[:],
                             start=True, stop=True)
            ab = st.tile([P, 2], F32, name="ab")
            nc.vector.tensor_copy(ab[:], psum_b[:])
            inv_a = ab[:, 0:1]   # [128,1] broadcast of 1/alpha
            al127 = ab[:, 1:2]   # [128,1] broadcast of alpha/127

            if not nw_ones:
                nw_b = cst.tile([P, K], F32, name="nw_b")
                nw_sb = cst.tile([1, K], F32, name="nw_sb")
                nc.scalar.dma_start(nw_sb[:], nwt.ap().rearrange("k -> 1 k"))
                for c in range(K // OCW):
                    pb = pacc_p.tile([P, OCW], F32, name="pacc")
                    nc.tensor.matmul(pb[:], lhsT=ones_row[:],
                                     rhs=nw_sb[:, c * OCW:(c + 1) * OCW],
                                     start=True, stop=True)
                    nc.vector.tensor_copy(nw_b[:, c * OCW:(c + 1) * OCW],
                                          pb[:])

            # ---------------- x pipeline ----------------
            xqt = xqt_p.tile([P, N_K, T_C], BF16, name="xqt")
            sy = [None] * N_T

            def x_phase(tt):
                xt = xin_p.tile([P, K], F32, name="xin")
                nc.sync.dma_start(xt[:], xs_r[tt])
                sq = scr_p.tile([P, K], BF16, name="scr")
                ssum = st.tile([P, 1], F32, name=f"ssum{tt}")
                nc.scalar.activation(sq[:], xt[:], ACTF.Square,
                                     accum_out=ssum[:])
                if not nw_ones:
                    nc.vector.tensor_tensor(xt[:], xt[:], nw_b[:], ALU.mult)
                amax = st.tile([P, 1], F32, name=f"amax{tt}")
                nc.vector.tensor_reduce(amax[:], xt[:],
                                        axis=mybir.AxisListType.X, op=ALU.max,
                                        apply_absolute_value=True)
                rms = st.tile([P, 1], F32, name=f"rms{tt}")
                nc.scalar.activation(rms[:], ssum[:], ACTF.Sqrt,
                                     scale=1.0 / K, bias=epsc[:])
                grd = st.tile([P, 1], F32, name=f"grd{tt}")
                nc.vector.tensor_scalar(grd[:], rms[:], 1e-10, None, ALU.mult)
                m = st.tile([P, 1], F32, name=f"m{tt}")
                nc.vector.tensor_tensor(m[:], amax[:], grd[:], ALU.max)
                m127 = st.tile([P, 1], F32, name=f"m127{tt}")
                nc.vector.tensor_scalar(m127[:], m[:], 1.0 / 127.0, None,
                                        ALU.mult)
                sA = st.tile([P, 1], F32, name=f"sA{tt}")
                nc.vector.reciprocal(sA[:], m127[:])
                # r = round(x * sA) + MAGIC   (in place over xt)
                nc.scalar.activation(xt[:], xt[:], ACTF.Identity,
                                     scale=sA[:], bias=magic[:])
                xq = xq_p.tile([P, K], BF16, name="xq")
                nc.vector.tensor_scalar(xq[:], xt[:], MAGIC, None,
                                        ALU.subtract)
                for g in range(N_K // 4):
                    pst = ptr_p.tile([P, 4 * P], BF16, name="ptr")
                    for j in range(4):
                        kk = 4 * g + j
                        nc.tensor.transpose(pst[:, j * P:(j + 1) * P],
                                            xq[:, kk * P:(kk + 1) * P],
                                            ident[:])
                    nc.vector.tensor_copy(
                        xqt[:, 4 * g:4 * g + 4, tt * P:(tt + 1) * P],
                        pst[:].rearrange("p (j c) -> p j c", j=4))
                # S_y = alpha * m / (127 * rms)
                rinv = st.tile([P, 1], F32, name=f"rinv{tt}")
                nc.vector.reciprocal(rinv[:], rms[:])
                t1 = st.tile([P, 1], F32, name=f"t1{tt}")
                nc.vector.tensor_scalar(t1[:], m[:], al127, None, ALU.mult)
                syt = st.tile([P, 1], F32, name=f"sy{tt}")
                nc.vector.tensor_tensor(syt[:], t1[:], rinv[:], ALU.mult)
                sy[tt] = syt

            # one quant chain covers TWO k-tiles (kk=2g, 2g+1) via a 3D AP:
            # halves the per-tile ACT cost so wq production outruns the PE.
            wt_pair = wt.ap().rearrange("(g j p) o -> g p j o", j=2, p=P)

            def w_quant_pair(oc, g):
                wf = wf_p.tile([P, 2, OCW], F32, name="wf")
                nc.sync.dma_start(
                    wf[:], wt_pair[g][:, :, oc * OCW:(oc + 1) * OCW])
                # r = round(w / alpha) + MAGIC   (in place)
                nc.scalar.activation(wf[:], wf[:], ACTF.Identity,
                                     scale=inv_a, bias=magic[:])
                q1 = q1_p.tile([P, 2, OCW], BF16, name="q1")
                nc.vector.tensor_scalar(q1[:], wf[:], MAGIC, 1.0,
                                        ALU.subtract, ALU.min)
                wq = wq_p.tile([P, 2, OCW], BF16, name="wq")
                nc.vector.tensor_scalar(wq[:], q1[:], -1.0, None, ALU.max)
                return wq

            def mm_phase(oc, wq_tiles):
                # kk-inner over tt half-groups: each wq tile's last reader is
                # early in the chunk.  During the second half-group, interleave
                # the NEXT chunk's quant chain right behind each freed slot so
                # wq production stays phase-aligned with consumption.
                nxt = []
                for h in range(N_T // TT_H):
                    tts = list(range(h * TT_H, (h + 1) * TT_H))
                    pas = {tt: pacc_p.tile([P, OCW], F32, name="pacc")
                           for tt in tts}
                    for kk in range(N_K):
                        g, j = kk // 2, kk % 2
                        for tt in tts:
                            nc.tensor.matmul(
                                pas[tt][:],
                                lhsT=xqt[:, kk, tt * P:(tt + 1) * P],
                                rhs=wq_tiles[g][:, j, :],
                                start=(kk == 0), stop=(kk == N_K - 1))
                        if h == 1 and j == 1 and oc + 1 < N_OC:
                            nxt.append(w_quant_pair(oc + 1, g))
                    for tt in tts:
                        yt = y_p.tile([P, OCW], F32, name="yo")
                        nc.scalar.activation(yt[:], pas[tt][:], ACTF.Identity,
                                             scale=sy[tt][:])
                        nc.sync.dma_start(
                            ys_a[tt * P:(tt + 1) * P,
                                 oc * OCW:(oc + 1) * OCW],
                            yt[:])
                return nxt

            # Emission order drives scheduling priority + DMA queue order.
            x_phase(0)
            wq_cur = [w_quant_pair(0, g) for g in range(N_K // 2)]
            for tt in range(1, N_T):
                x_phase(tt)
            for oc in range(N_OC):
                wq_cur = mm_phase(oc, wq_cur)
    _split_excess_waits(nc)
    return nc


_PROGRAMS: dict = {}


def _get_program(key):
    if key not in _PROGRAMS:
        if key == "alpha":
            _PROGRAMS[key] = build_alpha_program()
        else:
            _PROGRAMS[key] = build_main_program(key == "main_ones")
    return _PROGRAMS[key]


def kernel(x, weight, norm_weight, _trace=False, _trace_kwargs=None):
    x = np.ascontiguousarray(np.asarray(x, dtype=np.float32))
    W = np.asarray(weight, dtype=np.float32)
    nw = np.ascontiguousarray(np.asarray(norm_weight, dtype=np.float32))
    b, s, k = x.shape
    assert (b * s, k) == (4096, K) and W.shape == (4096, K)
    x2 = x.reshape(b * s, k)
    nw_ones = bool(np.all(nw == 1.0))
    # k-major shards of W (layout prep only -- no arithmetic)
    wts = [np.ascontiguousarray(W[O_C * j:O_C * (j + 1), :].T)
           for j in range(2)]

    kwargs = dict(trace=True, **(_trace_kwargs or {})) if _trace else {}

    # ---- launch 1: alpha partials over disjoint 1/8 slices of W ----
    nc_a = _get_program("alpha")
    in_a = []
    for c in range(N_CORES):
        i, j = c % 4, c // 4
        in_a.append({"wa": wts[j][T_C * i:T_C * (i + 1)]})
    res_a = bass_utils.run_bass_kernel_spmd(
        nc_a, in_a, core_ids=list(range(N_CORES)), **kwargs)
    total = np.float64(0.0)
    for c in range(N_CORES):
        total += np.float64(res_a.results[c]["apart"][0, 0])
    alpha = np.maximum(np.float32(np.float32(total) / np.float32(K * 4096)),
                       np.float32(1e-10))
    ab = np.array([[np.float32(1.0) / alpha, alpha / np.float32(127.0)]],
                  dtype=np.float32)

    # ---- launch 2: main kernel ----
    nc_m = _get_program("main_ones" if nw_ones else "main_gen")
    in_m = []
    for c in range(N_CORES):
        i, j = c % 4, c // 4
        in_m.append({"xs": x2[T_C * i:T_C * (i + 1)], "wt": wts[j],
                     "ab": ab, "nw": nw})
    res_m = bass_utils.run_bass_kernel_spmd(
        nc_m, in_m, core_ids=list(range(N_CORES)), **kwargs)

    y = np.empty((4096, 4096), dtype=np.float32)
    for c in range(N_CORES):
        i, j = c % 4, c // 4
        y[T_C * i:T_C * (i + 1), O_C * j:O_C * (j + 1)] = \
            res_m.results[c]["ys"]
    out = y.reshape(b, s, 4096)
    if _trace:
        return out, (res_a, res_m)
    return out


# revision 21
# speedup vs baseline: 2.2615x; 1.0489x over previous
"""FusedBitLinear Trainium2 kernel.

y = BitLinear(x, W, nw):
    rms   = sqrt(mean(x^2, -1) + 1e-6)
    x_n   = x / rms * nw
    alpha = max(mean(|W|), 1e-10)
    w_q   = clip(round(W / alpha), -1, 1)            (ternary)
    gamma = max(absmax(x_n, -1), 1e-10)
    x_q   = clip(round(x_n * 127 / gamma), -128, 127)
    y     = (x_q @ w_q.T) * (alpha * gamma / 127)

Key identities used on device:
    A[t]   = absmax(x[t,:] * nw)                     (per token)
    m[t]   = max(A[t], 1e-10 * rms[t])
    x_q    = round(x * nw * 127 / m[t])              (rms cancels; |..| <= 127 so
                                                      the clip never binds)
    y      = (x_q @ w_q.T) * alpha * m[t] / (127 * rms[t])
round() is the fp32 magic-add trick fused into an ACT fma (single rounding ->
exact round-to-nearest-even).  x_q in [-127,127] and ternary w_q are exact in
bf16, and 4096-long dot products of |v|<=127 integers fit fp32 PSUM exactly ->
the bf16 matmul is bit-exact.

Sharding (8 cores): 4 token-groups x 2 out-feature groups.  Each core gets
x rows [1024, 4096] and the k-major transpose of its W shard [4096, 2048].

Two launches: a tiny kernel reduces a disjoint 1/8 slice of |W| per core
(the only cross-core quantity), the host combines the 8 partials into
(1/alpha, alpha/127), and the main kernel takes those as a [1,2] input --
no collective on the main kernel's critical path.
"""

import numpy as np

import bass_rust as _bass_rust
import concourse.bass as bass
import concourse.mybir as mybir
import concourse.tile as tile
from concourse import bass_utils
from concourse.masks import make_identity
from concourse.vector_clock import ScopedClock, VectorClock

F32 = mybir.dt.float32
BF16 = mybir.dt.bfloat16
ALU = mybir.AluOpType
ACTF = mybir.ActivationFunctionType

N_CORES = 8
P = 128
K = 4096            # in_features
T_C = 1024          # tokens per core
O_C = 2048          # out features per core
N_T = T_C // P      # 8 token tiles
N_K = K // P        # 32 k tiles
OCW = 512           # out-feature chunk width (matmul moving free dim)
N_OC = O_C // OCW   # 4 chunks
TT_H = 4            # token tiles per half-group (psum banks per group)
MAGIC = 12582912.0  # 1.5 * 2**23 : fp32 round-to-nearest-even magic
NORM_EPS = 1e-6

_patched = False


def _patch_drain_and_barrier():
    """The walrus build in this env allows at most ~2 sync waits per
    instruction, but TileContext's exit drain piles one wait per logical
    processor onto a single Drain.  Split it: one drain per outstanding proc."""
    global _patched
    if _patched:
        return
    _patched = True

    def _drain_and_barrier(self, tick_clock, wait_clock):
        gvc = tick_clock.global_clock
        try:
            items = gvc.items()
        except AttributeError:
            items = [(None, gvc)]
        for scope, vc in items:
            for p in range(len(vc)):
                t = vc[p]
                if t <= 0:
                    continue
                part = VectorClock()
                part.require_at_least(p, t)
                d = self.nc.sync.drain()
                wait_clock.add_sem_waits(d.ins, ScopedClock({scope: part}))
        self.nc.all_engine_barrier()
        assert self.sems is not None
        popped = self.nc._tile_sem_poison_stack.pop()
        assert popped is self._sem_poison
        self.nc.clear_and_free_semaphores(list(self.sems.allocated().values()))
        self.nc.all_engine_barrier()

    tile.TileContext._drain_and_barrier = _drain_and_barrier


_MAX_WAITS = 1      # per-instruction wait slots walrus accepts (DMA: 1)
_EV_WAITS = 2       # EventSemaphore instructions can hold 2
_wsplit_n = [0]


def _split_excess_waits(nc: bass.Bass):
    """walrus rejects instructions with >1-2 sync waits.  Hoist the excess
    onto EventSemaphore instructions inserted immediately before, on the same
    engine (program order on that engine preserves the blocking semantics)."""
    for fn in nc.m.functions:
        for bb in fn.blocks:
            insts = bb.instructions
            out = []
            for ins in insts:
                si = ins.sync_info
                waits = list(si.on_wait) if si and si.on_wait else []
                if len(waits) > _MAX_WAITS:
                    keep = waits[-_MAX_WAITS:]
                    excess = waits[:-_MAX_WAITS]
                    for i in range(0, len(excess), _EV_WAITS):
                        ev = mybir.InstEventSemaphore(
                            name=f"wsplit-{_wsplit_n[0]}", ins=[], outs=[])
                        _wsplit_n[0] += 1
                        ev.engine = ins.engine
                        ev.sync_info = _bass_rust.SyncInfo(
                            on_wait=excess[i:i + _EV_WAITS], on_update=[])
                        out.append(ev)
                    ins.sync_info = _bass_rust.SyncInfo(
                        on_wait=keep,
                        on_update=list(si.on_update) if si.on_update else [])
                out.append(ins)
            insts[:] = out


def build_alpha_program() -> bass.Bass:
    """Per-core partial sum of |W| over a disjoint [1024, 2048] slice."""
    _patch_drain_and_barrier()
    nc = bass.Bass("TRN2", target_bir_lowering=False, debug=False,
                   enable_asserts=False, num_devices=N_CORES)
    wa = nc.dram_tensor("wa", [T_C, O_C], F32, kind="ExternalInput")
    ap_out = nc.dram_tensor("apart", [1, 1], F32, kind="ExternalOutput")
    wa_c = wa.ap().rearrange("(a b p) o -> a p b o", b=2, p=P)
    N_AC = 4
    with tile.TileContext(nc) as tc:
        with tc.tile_pool(name="sb", bufs=2) as sb, \
             tc.tile_pool(name="st", bufs=1) as st, \
             tc.tile_pool(name="ps", bufs=1, space="PSUM") as ps:
            ones_col = st.tile([P, 1], F32, name="ones_col")
            nc.gpsimd.memset(ones_col[:], 1.0)
            apart = st.tile([P, N_AC], F32, name="apart")
            for a in range(N_AC):
                at = sb.tile([P, 2, O_C], F32, name="aw")
                nc.sync.dma_start(at[:], wa_c[a])
                nc.vector.tensor_reduce(apart[:, a:a + 1], at[:],
                                        axis=mybir.AxisListType.XY, op=ALU.add,
                                        apply_absolute_value=True)
            asum = st.tile([P, 1], F32, name="asum")
            nc.vector.tensor_reduce(asum[:], apart[:],
                                    axis=mybir.AxisListType.X, op=ALU.add)
            psum_a = ps.tile([1, 1], F32, name="pss")
            nc.tensor.matmul(psum_a[:], lhsT=ones_col[:], rhs=asum[:],
                             start=True, stop=True)
            tsum = st.tile([1, 1], F32, name="tsum")
            nc.vector.tensor_copy(tsum[:], psum_a[:])
            nc.sync.dma_start(ap_out.ap(), tsum[:])
    _split_excess_waits(nc)
    return nc


def build_main_program(nw_ones: bool) -> bass.Bass:
    _patch_drain_and_barrier()
    nc = bass.Bass("TRN2", target_bir_lowering=False, debug=False,
                   enable_asserts=False, num_devices=N_CORES)
    xs = nc.dram_tensor("xs", [T_C, K], F32, kind="ExternalInput")
    wt = nc.dram_tensor("wt", [K, O_C], F32, kind="ExternalInput")
    abt = nc.dram_tensor("ab", [1, 2], F32, kind="ExternalInput")
    nwt = nc.dram_tensor("nw", [K], F32, kind="ExternalInput")
    ys = nc.dram_tensor("ys", [T_C, O_C], F32, kind="ExternalOutput")

    xs_r = xs.ap().rearrange("(a p) k -> a p k", p=P)
    wt_a = wt.ap()
    ys_a = ys.ap()

    with tile.TileContext(nc) as tc:
        with tc.tile_pool(name="const", bufs=1) as cst, \
             tc.tile_pool(name="stat", bufs=1) as st, \
             tc.tile_pool(name="xin", bufs=2) as xin_p, \
             tc.tile_pool(name="scr", bufs=1) as scr_p, \
             tc.tile_pool(name="xq", bufs=2) as xq_p, \
             tc.tile_pool(name="xqt", bufs=1) as xqt_p, \
             tc.tile_pool(name="wf", bufs=3) as wf_p, \
             tc.tile_pool(name="q1", bufs=3) as q1_p, \
             tc.tile_pool(name="wq", bufs=18) as wq_p, \
             tc.tile_pool(name="yo", bufs=3) as y_p, \
             tc.tile_pool(name="ptr", bufs=2, space="PSUM") as ptr_p, \
             tc.tile_pool(name="pacc", bufs=6, space="PSUM") as pacc_p:

            # ---------------- constants ----------------
            magic = cst.tile([P, 1], F32, name="magic")
            nc.gpsimd.memset(magic[:], MAGIC)
            epsc = cst.tile([P, 1], F32, name="epsc")
            nc.gpsimd.memset(epsc[:], NORM_EPS)
            ones_row = cst.tile([1, P], F32, name="ones_row")
            nc.gpsimd.memset(ones_row[:], 1.0)
            ident = cst.tile([P, P], BF16, name="ident")
            make_identity(nc, ident[:])

            # alpha scalars: ab = [1/alpha, alpha/127] -> broadcast to [128,2]
            ab_sb = cst.tile([1, 2], F32, name="ab_sb")
            nc.scalar.dma_start(ab_sb[:], abt.ap())
            psum_b = pacc_p.tile([P, OCW], F32, name="pacc")[:, 0:2]
            nc.tensor.matmul(psum_b[:], lhsT=ones_row[:], rhs=ab_sb[:],
                             start=True, stop=True)
            ab = st.tile([P, 2], F32, name="ab")
            nc.vector.tensor_copy(ab[:], psum_b[:])
            inv_a = ab[:, 0:1]   # [128,1] broadcast of 1/alpha
            al127 = ab[:, 1:2]   # [128,1] broadcast of alpha/127

            if not nw_ones:
                nw_b = cst.tile([P, K], F32, name="nw_b")
                nw_sb = cst.tile([1, K], F32, name="nw_sb")
                nc.scalar.dma_start(nw_sb[:], nwt.ap().rearrange("k -> 1 k"))
                for c in range(K // OCW):
                    pb = pacc_p.tile([P, OCW], F32, name="pacc")
                    nc.tensor.matmul(pb[:], lhsT=ones_row[:],
                                     rhs=nw_sb[:, c * OCW:(c + 1) * OCW],
                                     start=True, stop=True)
                    nc.vector.tensor_copy(nw_b[:, c * OCW:(c + 1) * OCW],
                                          pb[:])

            # ---------------- x pipeline ----------------
            xqt = xqt_p.tile([P, N_K, T_C], BF16, name="xqt")
            sy = [None] * N_T

            def x_phase(tt):
                xt = xin_p.tile([P, K], F32, name="xin")
                nc.sync.dma_start(xt[:], xs_r[tt])
                sq = scr_p.tile([P, K], BF16, name="scr")
                ssum = st.tile([P, 1], F32, name=f"ssum{tt}")
                nc.scalar.activation(sq[:], xt[:], ACTF.Square,
                                     accum_out=ssum[:])
                if not nw_ones:
                    nc.vector.tensor_tensor(xt[:], xt[:], nw_b[:], ALU.mult)
                amax = st.tile([P, 1], F32, name=f"amax{tt}")
                nc.vector.tensor_reduce(amax[:], xt[:],
                                        axis=mybir.AxisListType.X, op=ALU.max,
                                        apply_absolute_value=True)
                rms = st.tile([P, 1], F32, name=f"rms{tt}")
                nc.scalar.activation(rms[:], ssum[:], ACTF.Sqrt,
                                     scale=1.0 / K, bias=epsc[:])
                grd = st.tile([P, 1], F32, name=f"grd{tt}")
                nc.vector.tensor_scalar(grd[:], rms[:], 1e-10, None, ALU.mult)
                m = st.tile([P, 1], F32, name=f"m{tt}")
                nc.vector.tensor_tensor(m[:], amax[:], grd[:], ALU.max)
                m127 = st.tile([P, 1], F32, name=f"m127{tt}")
                nc.vector.tensor_scalar(m127[:], m[:], 1.0 / 127.0, None,
                                        ALU.mult)
                sA = st.tile([P, 1], F32, name=f"sA{tt}")
                nc.vector.reciprocal(sA[:], m127[:])
                # r = round(x * sA) + MAGIC   (in place over xt)
                nc.scalar.activation(xt[:], xt[:], ACTF.Identity,
                                     scale=sA[:], bias=magic[:])
                xq = xq_p.tile([P, K], BF16, name="xq")
                nc.vector.tensor_scalar(xq[:], xt[:], MAGIC, None,
                                        ALU.subtract)
                for g in range(N_K // 4):
                    pst = ptr_p.tile([P, 4 * P], BF16, name="ptr")
                    for j in range(4):
                        kk = 4 * g + j
                        nc.tensor.transpose(pst[:, j * P:(j + 1) * P],
                                            xq[:, kk * P:(kk + 1) * P],
                                            ident[:])
                    nc.vector.tensor_copy(
                        xqt[:, 4 * g:4 * g + 4, tt * P:(tt + 1) * P],
                        pst[:].rearrange("p (j c) -> p j c", j=4))
                # S_y = alpha * m / (127 * rms)
                rinv = st.tile([P, 1], F32, name=f"rinv{tt}")
                nc.vector.reciprocal(rinv[:], rms[:])
                t1 = st.tile([P, 1], F32, name=f"t1{tt}")
                nc.vector.tensor_scalar(t1[:], m[:], al127, None, ALU.mult)
                syt = st.tile([P, 1], F32, name=f"sy{tt}")
                nc.vector.tensor_tensor(syt[:], t1[:], rinv[:], ALU.mult)
                sy[tt] = syt

            # one quant chain covers TWO k-tiles (kk=2g, 2g+1) via a 3D AP:
            # halves the per-tile ACT cost so wq production outruns the PE.
            wt_pair = wt.ap().rearrange("(g j p) o -> g p j o", j=2, p=P)

            def w_quant_pair(oc, g):
                wf = wf_p.tile([P, 2, OCW], F32, name="wf")
                nc.sync.dma_start(
                    wf[:], wt_pair[g][:, :, oc * OCW:(oc + 1) * OCW])
                # r = round(w / alpha) + MAGIC   (in place)
                nc.scalar.activation(wf[:], wf[:], ACTF.Identity,
                                     scale=inv_a, bias=magic[:])
                q1 = q1_p.tile([P, 2, OCW], BF16, name="q1")
                nc.vector.tensor_scalar(q1[:], wf[:], MAGIC, 1.0,
                                        ALU.subtract, ALU.min)
                wq = wq_p.tile([P, 2, OCW], BF16, name="wq")
                nc.vector.tensor_scalar(wq[:], q1[:], -1.0, None, ALU.max)
                return wq

            def mm_phase(oc, wq_tiles, tt_h):
                # kk-inner over tt groups: each wq tile's last reader is early
                # in the chunk.  During the second half of the chunk's MM
                # stream, interleave the NEXT chunk's quant chains behind the
                # freed slots so wq production stays ahead of consumption.
                ngroups = N_T // tt_h
                npairs = N_K // 2
                slots_total = (ngroups - ngroups // 2) * npairs
                nxt = []
                for h in range(ngroups):
                    tts = list(range(h * tt_h, (h + 1) * tt_h))
                    pas = {tt: pacc_p.tile([P, OCW], F32, name="pacc")
                           for tt in tts}
                    for kk in range(N_K):
                        g, j = kk // 2, kk % 2
                        for tt in tts:
                            nc.tensor.matmul(
                                pas[tt][:],
                                lhsT=xqt[:, kk, tt * P:(tt + 1) * P],
                                rhs=wq_tiles[g][:, j, :],
                                start=(kk == 0), stop=(kk == N_K - 1))
                        if oc + 1 < N_OC and h >= ngroups // 2 and j == 1:
                            slot = (h - ngroups // 2) * npairs + g
                            want = (slot + 1) * npairs // slots_total
                            while len(nxt) < want:
                                nxt.append(w_quant_pair(oc + 1, len(nxt)))
                    for tt in tts:
                        yt = y_p.tile([P, OCW], F32, name="yo")
                        nc.vector.tensor_tensor(
                            yt[:], pas[tt][:],
                            sy[tt][:].to_broadcast((P, OCW)), ALU.mult)
                        nc.sync.dma_start(
                            ys_a[tt * P:(tt + 1) * P,
                                 oc * OCW:(oc + 1) * OCW],
                            yt[:])
                return nxt

            # Emission order drives scheduling priority + DMA queue order:
            # x tile 0 first, W chunk-0 pairs interleaved with early x tiles
            # so the first matmuls unblock ASAP.
            x_phase(0)
            wq_cur = [w_quant_pair(0, g) for g in range(8)]
            x_phase(1)
            wq_cur += [w_quant_pair(0, g) for g in range(8, N_K // 2)]
            for tt in range(2, N_T):
                x_phase(tt)
            for oc in range(N_OC):
                wq_cur = mm_phase(oc, wq_cur, TT_H if oc else 2)
    _split_excess_waits(nc)
    return nc


_PROGRAMS: dict = {}


def _get_program(key):
    if key not in _PROGRAMS:
        if key == "alpha":
            _PROGRAMS[key] = build_alpha_program()
        else:
            _PROGRAMS[key] = build_main_program(key == "main_ones")
    return _PROGRAMS[key]


def kernel(x, weight, norm_weight, _trace=False, _trace_kwargs=None):
    x = np.ascontiguousarray(np.asarray(x, dtype=np.float32))
    W = np.asarray(weight, dtype=np.float32)
    nw = np.ascontiguousarray(np.asarray(norm_weight, dtype=np.float32))
    b, s, k = x.shape
    assert (b * s, k) == (4096, K) and W.shape == (4096, K)
    x2 = x.reshape(b * s, k)
    nw_ones = bool(np.all(nw == 1.0))
    # k-major shards of W (layout prep only -- no arithmetic)
    wts = [np.ascontiguousarray(W[O_C * j:O_C * (j + 1), :].T)
           for j in range(2)]

    kwargs = dict(trace=True, **(_trace_kwargs or {})) if _trace else {}

    # ---- launch 1: alpha partials over disjoint 1/8 slices of W ----
    nc_a = _get_program("alpha")
    in_a = []
    for c in range(N_CORES):
        i, j = c % 4, c // 4
        in_a.append({"wa": wts[j][T_C * i:T_C * (i + 1)]})
    res_a = bass_utils.run_bass_kernel_spmd(
        nc_a, in_a, core_ids=list(range(N_CORES)), **kwargs)
    total = np.float64(0.0)
    for c in range(N_CORES):
        total += np.float64(res_a.results[c]["apart"][0, 0])
    alpha = np.maximum(np.float32(np.float32(total) / np.float32(K * 4096)),
                       np.float32(1e-10))
    ab = np.array([[np.float32(1.0) / alpha, alpha / np.float32(127.0)]],
                  dtype=np.float32)

    # ---- launch 2: main kernel ----
    nc_m = _get_program("main_ones" if nw_ones else "main_gen")
    in_m = []
    for c in range(N_CORES):
        i, j = c % 4, c // 4
        in_m.append({"xs": x2[T_C * i:T_C * (i + 1)], "wt": wts[j],
                     "ab": ab, "nw": nw})
    res_m = bass_utils.run_bass_kernel_spmd(
        nc_m, in_m, core_ids=list(range(N_CORES)), **kwargs)

    y = np.empty((4096, 4096), dtype=np.float32)
    for c in range(N_CORES):
        i, j = c % 4, c // 4
        y[T_C * i:T_C * (i + 1), O_C * j:O_C * (j + 1)] = \
            res_m.results[c]["ys"]
    out = y.reshape(b, s, 4096)
    if _trace:
        return out, (res_a, res_m)
    return out
